# revision 4
# baseline (speedup 1.0000x reference)
"""Trainium2 Bass kernel for nn_ClusteringLayer (vq codebook assign + gather).

Math (per reference): for each token t, idx = argmin_k ||c_k||^2 - 2 x_t . c_k,
y_t = centers[idx]. Output = stack([x, y]).

Strategy: data-parallel over tokens across 8 NeuronCores (batch axis shard,
codebook replicated). On each core, scores s = (2x).c - ||c||^2 are computed
on the PE with an exact bf16 hi/lo 3-term expansion (xh.ch + xh.cl + xl.ch,
fp32 PSUM accumulation), which reproduces fp32 argmin decisions for this
distribution (verified: 0 argmin flips vs fp64 on the full input set, while
1-pass bf16 flips 135). Per 128-token tile: 8 PSUM banks of [128,512] scores,
DVE evacuates (psum - c2) to SBUF, a max/max_index pair finds the argmax
column, and an indirect DMA gathers centers rows into y.
"""

import numpy as np
import ml_dtypes

import concourse.bass as bass
import concourse.bacc as bacc
import concourse.mybir as mybir
import concourse.tile as tile
from concourse.bass_utils import run_bass_kernel_spmd

B, T, D, K = 8, 4096, 512, 4096
NCORES = 8
TOK = (B * T) // NCORES      # tokens per core
P = 128                      # partitions / tokens per tile
NBANK = K // 512             # psum banks per token tile (8)
DCH = D // P                 # contraction chunks (4)
NEG_INF = -3.0e38

_PROGRAM_CACHE = {}

# test.py introspection: holds the BassKernelResults of the last run
LAST_RUN = {}


def _build_program(ttiles):
    dt = mybir.dt
    nc = bacc.Bacc("TRN2", target_bir_lowering=False, debug=False,
                   num_devices=NCORES)
    ntok = ttiles * P
    xh_d = nc.dram_tensor("xh", [D, ntok], dt.bfloat16, kind="ExternalInput").ap()
    xl_d = nc.dram_tensor("xl", [D, ntok], dt.bfloat16, kind="ExternalInput").ap()
    ch_d = nc.dram_tensor("ch", [D, K], dt.bfloat16, kind="ExternalInput").ap()
    cl_d = nc.dram_tensor("cl", [D, K], dt.bfloat16, kind="ExternalInput").ap()
    c2_d = nc.dram_tensor("c2", [P, K], dt.float32, kind="ExternalInput").ap()
    cent_d = nc.dram_tensor("cent", [K, D], dt.float32, kind="ExternalInput").ap()
    y_d = nc.dram_tensor("y", [ntok, D], dt.float32, kind="ExternalOutput").ap()

    with tile.TileContext(nc) as tc:
        with tc.tile_pool(name="const", bufs=1) as cpool, \
             tc.tile_pool(name="work", bufs=2) as wpool, \
             tc.tile_pool(name="psum", bufs=1, space="PSUM") as ppool:
            ch_sb = []
            cl_sb = []
            for d in range(DCH):
                t_ch = cpool.tile([P, K], dt.bfloat16, tag=f"ch{d}", name=f"ch{d}")
                nc.sync.dma_start(out=t_ch, in_=ch_d[d * P:(d + 1) * P, :])
                ch_sb.append(t_ch)
                t_cl = cpool.tile([P, K], dt.bfloat16, tag=f"cl{d}", name=f"cl{d}")
                nc.sync.dma_start(out=t_cl, in_=cl_d[d * P:(d + 1) * P, :])
                cl_sb.append(t_cl)
            c2_sb = cpool.tile([P, K], dt.float32, tag="c2", name="c2sb")
            nc.sync.dma_start(out=c2_sb, in_=c2_d[:, :])

            for t in range(ttiles):
                xh_t = wpool.tile([P, DCH, P], dt.bfloat16, tag="xh",
                                  name=f"xh{t}", bufs=3)
                nc.sync.dma_start(
                    out=xh_t,
                    in_=xh_d[:, t * P:(t + 1) * P].rearrange(
                        "(c p) f -> p c f", p=P))
                xl_t = wpool.tile([P, DCH, P], dt.bfloat16, tag="xl",
                                  name=f"xl{t}", bufs=3)
                nc.sync.dma_start(
                    out=xl_t,
                    in_=xl_d[:, t * P:(t + 1) * P].rearrange(
                        "(c p) f -> p c f", p=P))

                scores = wpool.tile([P, K], dt.float32, tag="scores",
                                    name=f"sc{t}", bufs=2)
                maxb = wpool.tile([P, 8], dt.float32, tag="maxb",
                                  name=f"maxb{t}", bufs=2)
                idx8 = wpool.tile([P, 8], dt.uint32, tag="idx",
                                  name=f"idx{t}", bufs=2)
                ytile = wpool.tile([P, D], dt.float32, tag="yt",
                                   name=f"yt{t}", bufs=3)

                for n in range(NBANK):
                    ps = ppool.tile([P, 512], dt.float32, tag=f"ps{n}",
                                    name=f"ps{t}_{n}")
                    first = True
                    for (xlo, clo) in ((0, 0), (0, 1), (1, 0)):
                        xt = xl_t if xlo else xh_t
                        csb = cl_sb if clo else ch_sb
                        for d in range(DCH):
                            nc.tensor.matmul(
                                ps,
                                lhsT=xt[:, d, :],
                                rhs=csb[d][:, n * 512:(n + 1) * 512],
                                start=first,
                                stop=(xlo == 1 and d == DCH - 1),
                            )
                            first = False
                    nc.vector.tensor_tensor(
                        out=scores[:, n * 512:(n + 1) * 512],
                        in0=ps,
                        in1=c2_sb[:, n * 512:(n + 1) * 512],
                        op=mybir.AluOpType.subtract,
                    )
                nc.vector.max(out=maxb, in_=scores)
                nc.vector.max_index(out=idx8, in_max=maxb, in_values=scores)
                nc.gpsimd.indirect_dma_start(
                    out=ytile,
                    out_offset=None,
                    in_=cent_d,
                    in_offset=bass.IndirectOffsetOnAxis(ap=idx8[:, 0:1], axis=0),
                )
                nc.sync.dma_start(out=y_d[t * P:(t + 1) * P, :], in_=ytile)

    nc.compile()
    return nc


def _get_program(ttiles):
    if ttiles not in _PROGRAM_CACHE:
        _PROGRAM_CACHE[ttiles] = _build_program(ttiles)
    return _PROGRAM_CACHE[ttiles]


def _prep_inputs(x, centers, ntok_per_core, ncores):
    bf16 = ml_dtypes.bfloat16
    flat = np.ascontiguousarray(np.asarray(x, dtype=np.float32).reshape(-1, D))
    c = np.ascontiguousarray(np.asarray(centers, dtype=np.float32))

    ch = c.astype(bf16)
    cl = (c - ch.astype(np.float32)).astype(bf16)
    chT = np.ascontiguousarray(ch.T)
    clT = np.ascontiguousarray(cl.T)
    c2 = (c * c).sum(axis=-1, dtype=np.float32)
    c2b = np.ascontiguousarray(np.broadcast_to(c2[None, :], (P, K)))

    in_maps = []
    for i in range(ncores):
        xs = flat[i * ntok_per_core:(i + 1) * ntok_per_core]
        x2 = 2.0 * xs  # exact in fp32
        xh = x2.astype(bf16)
        xl = (x2 - xh.astype(np.float32)).astype(bf16)
        in_maps.append({
            "xh": np.ascontiguousarray(xh.T),
            "xl": np.ascontiguousarray(xl.T),
            "ch": chT,
            "cl": clT,
            "c2": c2b,
            "cent": c,
        })
    return in_maps


def kernel(x, centers):
    x = np.asarray(x, dtype=np.float32)
    nc = _get_program(TOK // P)
    in_maps = _prep_inputs(x, centers, TOK, NCORES)
    res = run_bass_kernel_spmd(nc, in_maps, core_ids=list(range(NCORES)))
    LAST_RUN["res"] = res
    y = np.concatenate([r["y"] for r in res.results], axis=0).reshape(x.shape)
    return np.stack([x, y], axis=0)


# revision 8
# speedup vs baseline: 1.0117x; 1.0117x over previous
"""Trainium2 Bass kernel for nn_ClusteringLayer (vq codebook assign + gather).

Math (per reference): for each token t, idx = argmin_k ||c_k||^2 - 2 x_t . c_k,
y_t = centers[idx]. Output = stack([x, y]).

Strategy: data-parallel over tokens across 8 NeuronCores (batch axis shard,
codebook replicated). On each core, scores s = (2x).c - ||c||^2 are computed
on the PE with an exact bf16 hi/lo 3-term expansion (xh.ch + xh.cl + xl.ch,
fp32 PSUM accumulation), which reproduces fp32 argmin decisions for this
distribution (verified: 0 argmin flips vs fp64 on the full input set, while
1-pass bf16 flips 135). Per 128-token tile: 8 PSUM banks of [128,512] scores,
DVE evacuates (psum - c2) to SBUF, a max/max_index pair finds the argmax
column, and an indirect DMA gathers centers rows into y.
"""

import numpy as np
import ml_dtypes

import concourse.bass as bass
import concourse.bacc as bacc
import concourse.mybir as mybir
import concourse.tile as tile
from concourse.bass_utils import run_bass_kernel_spmd

B, T, D, K = 8, 4096, 512, 4096
NCORES = 8
TOK = (B * T) // NCORES      # tokens per core
P = 128                      # partitions / tokens per tile
NBANK = K // 512             # psum banks per token tile (8)
DCH = D // P                 # contraction chunks (4)
NEG_INF = -3.0e38

_PROGRAM_CACHE = {}

# test.py introspection: holds the BassKernelResults of the last run
LAST_RUN = {}


def _build_program(ttiles):
    dt = mybir.dt
    nc = bacc.Bacc("TRN2", target_bir_lowering=False, debug=False,
                   num_devices=NCORES)
    ntok = ttiles * P
    xh_d = nc.dram_tensor("xh", [D, ntok], dt.bfloat16, kind="ExternalInput").ap()
    xl_d = nc.dram_tensor("xl", [D, ntok], dt.bfloat16, kind="ExternalInput").ap()
    ch_d = nc.dram_tensor("ch", [D, K], dt.bfloat16, kind="ExternalInput").ap()
    cl_d = nc.dram_tensor("cl", [D, K], dt.bfloat16, kind="ExternalInput").ap()
    c2_d = nc.dram_tensor("c2", [P, K], dt.float32, kind="ExternalInput").ap()
    cent_d = nc.dram_tensor("cent", [K, D], dt.float32, kind="ExternalInput").ap()
    y_d = nc.dram_tensor("y", [ntok, D], dt.float32, kind="ExternalOutput").ap()

    with tile.TileContext(nc) as tc:
        with tc.tile_pool(name="const", bufs=1) as cpool, \
             tc.tile_pool(name="work", bufs=2) as wpool, \
             tc.tile_pool(name="psum", bufs=1, space="PSUM") as ppool:
            # PE warmup: ~40 tiny matmuls keep the PE busy while the codebook
            # streams in, so the HAM clock-gate is already released (2.4 GHz)
            # when the first real matmul issues.
            warm = cpool.tile([P, 64], dt.bfloat16, tag="warm", name="warm")
            nc.vector.memset(warm, 0.0)
            # reuse the ps0 bank slot; released before the first real tile
            ps_warm = ppool.tile([P, 64], dt.float32, tag="ps0", name="pswarm")
            for w in range(40):
                nc.tensor.matmul(ps_warm[:64, :], lhsT=warm, rhs=warm,
                                 start=True, stop=True)

            def load_x_tile(t):
                xh_t = wpool.tile([P, DCH, P], dt.bfloat16, tag="xh",
                                  name=f"xh{t}", bufs=3)
                nc.sync.dma_start(
                    out=xh_t,
                    in_=xh_d[:, t * P:(t + 1) * P].rearrange(
                        "(c p) f -> p c f", p=P))
                xl_t = wpool.tile([P, DCH, P], dt.bfloat16, tag="xl",
                                  name=f"xl{t}", bufs=3)
                nc.sync.dma_start(
                    out=xl_t,
                    in_=xl_d[:, t * P:(t + 1) * P].rearrange(
                        "(c p) f -> p c f", p=P))
                return xh_t, xl_t

            # x tiles for the first two iterations load ahead of the bulky
            # codebook preload so bank-0 compute is not queued behind it
            x_pre = {t: load_x_tile(t) for t in range(min(2, ttiles))}

            # Preload codebook tiles sliced per 512-center bank, in bank
            # order, so bank-0 matmuls can start after ~1 MB instead of
            # waiting for the whole 10 MB preload.
            ch_sb = []
            cl_sb = []
            for d in range(DCH):
                t_ch = cpool.tile([P, K], dt.bfloat16, tag=f"ch{d}", name=f"ch{d}")
                ch_sb.append(t_ch)
                t_cl = cpool.tile([P, K], dt.bfloat16, tag=f"cl{d}", name=f"cl{d}")
                cl_sb.append(t_cl)
            c2_sb = cpool.tile([P, K], dt.float32, tag="c2", name="c2sb")
            for n in range(NBANK):
                cols = slice(n * 512, (n + 1) * 512)
                for d in range(DCH):
                    nc.sync.dma_start(out=ch_sb[d][:, cols],
                                      in_=ch_d[d * P:(d + 1) * P, cols])
                    nc.sync.dma_start(out=cl_sb[d][:, cols],
                                      in_=cl_d[d * P:(d + 1) * P, cols])
                nc.sync.dma_start(out=c2_sb[:, cols], in_=c2_d[:, cols])

            for t in range(ttiles):
                if t in x_pre:
                    xh_t, xl_t = x_pre.pop(t)
                else:
                    xh_t, xl_t = load_x_tile(t)

                scores = wpool.tile([P, K], dt.float32, tag="scores",
                                    name=f"sc{t}", bufs=2)
                maxb = wpool.tile([P, 8], dt.float32, tag="maxb",
                                  name=f"maxb{t}", bufs=2)
                idx8 = wpool.tile([P, 8], dt.uint32, tag="idx",
                                  name=f"idx{t}", bufs=2)
                ytile = wpool.tile([P, D], dt.float32, tag="yt",
                                   name=f"yt{t}", bufs=3)

                for n in range(NBANK):
                    ps = ppool.tile([P, 512], dt.float32, tag=f"ps{n}",
                                    name=f"ps{t}_{n}")
                    first = True
                    for (xlo, clo) in ((0, 0), (0, 1), (1, 0)):
                        xt = xl_t if xlo else xh_t
                        csb = cl_sb if clo else ch_sb
                        for d in range(DCH):
                            nc.tensor.matmul(
                                ps,
                                lhsT=xt[:, d, :],
                                rhs=csb[d][:, n * 512:(n + 1) * 512],
                                start=first,
                                stop=(xlo == 1 and d == DCH - 1),
                            )
                            first = False
                    nc.vector.tensor_tensor(
                        out=scores[:, n * 512:(n + 1) * 512],
                        in0=ps,
                        in1=c2_sb[:, n * 512:(n + 1) * 512],
                        op=mybir.AluOpType.subtract,
                    )
                nc.vector.max(out=maxb, in_=scores)
                nc.vector.max_index(out=idx8, in_max=maxb, in_values=scores)
                nc.gpsimd.indirect_dma_start(
                    out=ytile,
                    out_offset=None,
                    in_=cent_d,
                    in_offset=bass.IndirectOffsetOnAxis(ap=idx8[:, 0:1], axis=0),
                )
                nc.sync.dma_start(out=y_d[t * P:(t + 1) * P, :], in_=ytile)

    nc.compile()
    return nc


def _get_program(ttiles):
    if ttiles not in _PROGRAM_CACHE:
        _PROGRAM_CACHE[ttiles] = _build_program(ttiles)
    return _PROGRAM_CACHE[ttiles]


def _prep_inputs(x, centers, ntok_per_core, ncores):
    bf16 = ml_dtypes.bfloat16
    flat = np.ascontiguousarray(np.asarray(x, dtype=np.float32).reshape(-1, D))
    c = np.ascontiguousarray(np.asarray(centers, dtype=np.float32))

    ch = c.astype(bf16)
    cl = (c - ch.astype(np.float32)).astype(bf16)
    chT = np.ascontiguousarray(ch.T)
    clT = np.ascontiguousarray(cl.T)
    c2 = (c * c).sum(axis=-1, dtype=np.float32)
    c2b = np.ascontiguousarray(np.broadcast_to(c2[None, :], (P, K)))

    in_maps = []
    for i in range(ncores):
        xs = flat[i * ntok_per_core:(i + 1) * ntok_per_core]
        x2 = 2.0 * xs  # exact in fp32
        xh = x2.astype(bf16)
        xl = (x2 - xh.astype(np.float32)).astype(bf16)
        in_maps.append({
            "xh": np.ascontiguousarray(xh.T),
            "xl": np.ascontiguousarray(xl.T),
            "ch": chT,
            "cl": clT,
            "c2": c2b,
            "cent": c,
        })
    return in_maps


def kernel(x, centers):
    x = np.asarray(x, dtype=np.float32)
    nc = _get_program(TOK // P)
    in_maps = _prep_inputs(x, centers, TOK, NCORES)
    res = run_bass_kernel_spmd(nc, in_maps, core_ids=list(range(NCORES)))
    LAST_RUN["res"] = res
    y = np.concatenate([r["y"] for r in res.results], axis=0).reshape(x.shape)
    return np.stack([x, y], axis=0)


# revision 12
# speedup vs baseline: 1.0197x; 1.0080x over previous
"""Trainium2 Bass kernel for nn_ClusteringLayer (vq codebook assign + gather).

Math (per reference): for each token t, idx = argmin_k ||c_k||^2 - 2 x_t . c_k,
y_t = centers[idx]. Output = stack([x, y]).

Strategy: data-parallel over tokens across 8 NeuronCores (batch axis shard,
codebook replicated). On each core, scores s = (2x).c - ||c||^2 are computed
on the PE with an exact bf16 hi/lo 3-term expansion (xh.ch + xh.cl + xl.ch,
fp32 PSUM accumulation), which reproduces fp32 argmin decisions for this
distribution (verified: 0 argmin flips vs fp64 on the full input set, while
1-pass bf16 flips 135). Per 128-token tile: 8 PSUM banks of [128,512] scores,
DVE evacuates (psum - c2) to SBUF, a max/max_index pair finds the argmax
column, and an indirect DMA gathers centers rows into y.
"""

import numpy as np
import ml_dtypes

import concourse.bass as bass
import concourse.bacc as bacc
import concourse.mybir as mybir
import concourse.tile as tile
from concourse.bass_utils import run_bass_kernel_spmd

B, T, D, K = 8, 4096, 512, 4096
NCORES = 8
TOK = (B * T) // NCORES      # tokens per core
P = 128                      # partitions / tokens per tile
NBANK = K // 512             # psum banks per token tile (8)
DCH = D // P                 # contraction chunks (4)
NEG_INF = -3.0e38

_PROGRAM_CACHE = {}

# test.py introspection: holds the BassKernelResults of the last run
LAST_RUN = {}


def _build_program(ttiles):
    dt = mybir.dt
    nc = bacc.Bacc("TRN2", target_bir_lowering=False, debug=False,
                   num_devices=NCORES)
    ntok = ttiles * P
    xh_d = nc.dram_tensor("xh", [D, ntok], dt.bfloat16, kind="ExternalInput").ap()
    xl_d = nc.dram_tensor("xl", [D, ntok], dt.bfloat16, kind="ExternalInput").ap()
    ch_d = nc.dram_tensor("ch", [D, K], dt.bfloat16, kind="ExternalInput").ap()
    cl_d = nc.dram_tensor("cl", [D, K], dt.bfloat16, kind="ExternalInput").ap()
    c2_d = nc.dram_tensor("c2", [P, K], dt.float32, kind="ExternalInput").ap()
    cent_d = nc.dram_tensor("cent", [K, D], dt.float32, kind="ExternalInput").ap()
    y_d = nc.dram_tensor("y", [ntok, D], dt.float32, kind="ExternalOutput").ap()

    with tile.TileContext(nc) as tc:
        with tc.tile_pool(name="const", bufs=1) as cpool, \
             tc.tile_pool(name="work", bufs=2) as wpool, \
             tc.tile_pool(name="psum", bufs=1, space="PSUM") as ppool:
            def load_x_tile(t):
                xh_t = wpool.tile([P, DCH, P], dt.bfloat16, tag="xh",
                                  name=f"xh{t}", bufs=3)
                nc.sync.dma_start(
                    out=xh_t,
                    in_=xh_d[:, t * P:(t + 1) * P].rearrange(
                        "(c p) f -> p c f", p=P))
                xl_t = wpool.tile([P, DCH, P], dt.bfloat16, tag="xl",
                                  name=f"xl{t}", bufs=3)
                nc.sync.dma_start(
                    out=xl_t,
                    in_=xl_d[:, t * P:(t + 1) * P].rearrange(
                        "(c p) f -> p c f", p=P))
                return xh_t, xl_t

            # x tiles for the first two iterations load ahead of the bulky
            # codebook preload so bank-0 compute is not queued behind it
            x_pre = {t: load_x_tile(t) for t in range(min(2, ttiles))}

            # PE warmup: dense N=512 matmuls on the (tiny, early) t=0 x tile
            # keep the PE busy while the codebook streams in, so the HAM
            # clock-gate is released (2.4 GHz) before the real stream starts.
            # Results are garbage and never read; bank slot ps7 is needed
            # last by the real tile-0 work, so no WAR stall.
            ps_warm = ppool.tile([P, 512], dt.float32, tag="ps7",
                                 name="pswarm")
            warm_src = x_pre[0][0]
            for w in range(10):
                nc.tensor.matmul(ps_warm, lhsT=warm_src[:, 0, :],
                                 rhs=warm_src, start=True, stop=True)

            # Preload codebook tiles sliced per 512-center bank, in bank
            # order, so bank-0 matmuls can start after ~1 MB instead of
            # waiting for the whole 10 MB preload.
            ch_sb = []
            cl_sb = []
            for d in range(DCH):
                t_ch = cpool.tile([P, K], dt.bfloat16, tag=f"ch{d}", name=f"ch{d}")
                ch_sb.append(t_ch)
                t_cl = cpool.tile([P, K], dt.bfloat16, tag=f"cl{d}", name=f"cl{d}")
                cl_sb.append(t_cl)
            c2_sb = cpool.tile([P, K], dt.float32, tag="c2", name="c2sb")
            for n in range(NBANK):
                cols = slice(n * 512, (n + 1) * 512)
                for d in range(DCH):
                    nc.sync.dma_start(out=ch_sb[d][:, cols],
                                      in_=ch_d[d * P:(d + 1) * P, cols])
                    nc.sync.dma_start(out=cl_sb[d][:, cols],
                                      in_=cl_d[d * P:(d + 1) * P, cols])
                nc.sync.dma_start(out=c2_sb[:, cols], in_=c2_d[:, cols])

            for t in range(ttiles):
                if t in x_pre:
                    xh_t, xl_t = x_pre.pop(t)
                else:
                    xh_t, xl_t = load_x_tile(t)

                scores = wpool.tile([P, K], dt.float32, tag="scores",
                                    name=f"sc{t}", bufs=2)
                maxh1 = wpool.tile([P, 8], dt.float32, tag="maxh1",
                                   name=f"maxh1_{t}", bufs=2)
                maxh2 = wpool.tile([P, 8], dt.float32, tag="maxh2",
                                   name=f"maxh2_{t}", bufs=2)
                idxh1 = wpool.tile([P, 8], dt.uint32, tag="idxh1",
                                   name=f"idxh1_{t}", bufs=2)
                idxh2 = wpool.tile([P, 8], dt.uint32, tag="idxh2",
                                   name=f"idxh2_{t}", bufs=2)
                mask = wpool.tile([P, 1], dt.uint32, tag="mask",
                                  name=f"mask{t}", bufs=2)
                idxsel = wpool.tile([P, 1], dt.uint32, tag="idxsel",
                                    name=f"idxsel{t}", bufs=2)
                ytile = wpool.tile([P, D], dt.float32, tag="yt",
                                   name=f"yt{t}", bufs=3)
                half = NBANK // 2  # banks per argmax half

                for n in range(NBANK):
                    ps = ppool.tile([P, 512], dt.float32, tag=f"ps{n}",
                                    name=f"ps{t}_{n}")
                    first = True
                    for (xlo, clo) in ((0, 0), (0, 1), (1, 0)):
                        xt = xl_t if xlo else xh_t
                        csb = cl_sb if clo else ch_sb
                        for d in range(DCH):
                            nc.tensor.matmul(
                                ps,
                                lhsT=xt[:, d, :],
                                rhs=csb[d][:, n * 512:(n + 1) * 512],
                                start=first,
                                stop=(xlo == 1 and d == DCH - 1),
                            )
                            first = False
                    nc.vector.tensor_tensor(
                        out=scores[:, n * 512:(n + 1) * 512],
                        in0=ps,
                        in1=c2_sb[:, n * 512:(n + 1) * 512],
                        op=mybir.AluOpType.subtract,
                    )
                    if n == half - 1:
                        # first-half argmax overlaps banks 4-7 compute
                        nc.vector.max(out=maxh1, in_=scores[:, :half * 512])
                        nc.vector.max_index(out=idxh1, in_max=maxh1,
                                            in_values=scores[:, :half * 512])
                # second-half argmax + cross-half select
                nc.vector.max(out=maxh2, in_=scores[:, half * 512:])
                nc.vector.max_index(out=idxh2, in_max=maxh2,
                                    in_values=scores[:, half * 512:])
                nc.vector.tensor_scalar(
                    out=idxsel, in0=idxh2[:, 0:1], scalar1=half * 512,
                    scalar2=None, op0=mybir.AluOpType.add)
                nc.vector.tensor_tensor(
                    out=mask, in0=maxh1[:, 0:1], in1=maxh2[:, 0:1],
                    op=mybir.AluOpType.is_ge)
                nc.vector.copy_predicated(
                    out=idxsel, mask=mask, data=idxh1[:, 0:1])
                nc.gpsimd.indirect_dma_start(
                    out=ytile,
                    out_offset=None,
                    in_=cent_d,
                    in_offset=bass.IndirectOffsetOnAxis(ap=idxsel, axis=0),
                )
                nc.sync.dma_start(out=y_d[t * P:(t + 1) * P, :], in_=ytile)

    nc.compile()
    return nc


def _get_program(ttiles):
    if ttiles not in _PROGRAM_CACHE:
        _PROGRAM_CACHE[ttiles] = _build_program(ttiles)
    return _PROGRAM_CACHE[ttiles]


def _prep_inputs(x, centers, ntok_per_core, ncores):
    bf16 = ml_dtypes.bfloat16
    flat = np.ascontiguousarray(np.asarray(x, dtype=np.float32).reshape(-1, D))
    c = np.ascontiguousarray(np.asarray(centers, dtype=np.float32))

    ch = c.astype(bf16)
    cl = (c - ch.astype(np.float32)).astype(bf16)
    chT = np.ascontiguousarray(ch.T)
    clT = np.ascontiguousarray(cl.T)
    c2 = (c * c).sum(axis=-1, dtype=np.float32)
    c2b = np.ascontiguousarray(np.broadcast_to(c2[None, :], (P, K)))

    in_maps = []
    for i in range(ncores):
        xs = flat[i * ntok_per_core:(i + 1) * ntok_per_core]
        x2 = 2.0 * xs  # exact in fp32
        xh = x2.astype(bf16)
        xl = (x2 - xh.astype(np.float32)).astype(bf16)
        in_maps.append({
            "xh": np.ascontiguousarray(xh.T),
            "xl": np.ascontiguousarray(xl.T),
            "ch": chT,
            "cl": clT,
            "c2": c2b,
            "cent": c,
        })
    return in_maps


def kernel(x, centers):
    x = np.asarray(x, dtype=np.float32)
    nc = _get_program(TOK // P)
    in_maps = _prep_inputs(x, centers, TOK, NCORES)
    res = run_bass_kernel_spmd(nc, in_maps, core_ids=list(range(NCORES)))
    LAST_RUN["res"] = res
    y = np.concatenate([r["y"] for r in res.results], axis=0).reshape(x.shape)
    return np.stack([x, y], axis=0)
